# revision 9
# baseline (speedup 1.0000x reference)
"""Self-contained Trainium2 Bass kernel for a 6-layer post-LN transformer
encoder (B=2, S=2048, D=1024, H=16, F=4096, V=32000).

Sharding: sequence-parallel across 8 NeuronCores. Core c handles batch
b = c // 4, sequence slice [lc*512, (lc+1)*512) with lc = c % 4. Per layer,
one AllGather (replica groups [[0..3],[4..7]]) shares bf16 K/V across the
4 cores of each batch; all other compute is local to the core.

On-chip layout: residual stream is feature-major [1024 feats -> 8 chunks of
128 partitions, 512 tokens free] in f32. Matmul inputs are cast to bf16;
PSUM accumulation is f32. Attention scores are computed pre-transposed
(K @ Q^T per head) so softmax exp needs no transposes; the softmax
denominator comes from a ones-column appended to V in the AV matmul.
LayerNorm stats use ones-vector matmuls over the partition dim; rsqrt is
computed as exp(-0.5*ln(var+eps)) to stay in one ACT table set.
"""

import math
from contextlib import ExitStack

import ml_dtypes
import numpy as np

import concourse.bass as bass
import concourse.tile as tile
from concourse import bacc, mybir
from concourse.bass import IndirectOffsetOnAxis
from concourse.bass_utils import run_bass_kernel_spmd

dt = mybir.dt
Alu = mybir.AluOpType
Act = mybir.ActivationFunctionType

V, L, D, S, F, H, B = 32000, 6, 1024, 2048, 4096, 16, 2
DK = D // H          # 64
NC = 8               # cores
NT = (B * S) // NC   # 512 tokens per core
CH = D // 128        # 8 feature chunks
FCH = F // 128       # 32 ffn chunks
TC = NT // 128       # 4 token chunks per core
KCH = S // 128       # 16 key chunks per batch
EPS = 1e-5
SQRT_D = math.sqrt(D)
KV_HALF = D * NT     # flat elems of K (or V) contribution per core


def _ln(ctx, nc, psum, xres, xbfp, stats, x_in, g_sb, be_sb, ones_col_bf,
        ones_row_f32, eps_sb, name):
    """LayerNorm over features (partition dim) of feature-major x_in
    [128, CH, 512] f32. Returns (x_out f32 tile, x_out bf16 tile)."""
    # bf16 copies for stats matmuls
    x_b = xbfp.tile([128, CH, NT], dt.bfloat16, tag="sq", bufs=1,
                    name=f"xb_{name}")
    nc.vector.tensor_copy(out=x_b[:, :, :], in_=x_in[:, :, :])
    x_sq = xbfp.tile([128, CH, NT], dt.bfloat16, tag="sq2", bufs=1,
                     name=f"xsq_{name}")
    nc.vector.tensor_tensor(out=x_sq[:, :, :], in0=x_b[:, :, :],
                            in1=x_b[:, :, :], op=Alu.mult)
    # column sums via ones-matmul, accumulated over feature chunks
    st_sum = psum.tile([1, NT], dt.float32, tag="acc", name=f"sts_{name}")
    for kc in range(CH):
        nc.tensor.matmul(st_sum[:, :], lhsT=ones_col_bf[:, :],
                         rhs=x_b[:, kc, :], start=(kc == 0), stop=(kc == CH - 1))
    st_sq = psum.tile([1, NT], dt.float32, tag="acc", name=f"stq_{name}")
    for kc in range(CH):
        nc.tensor.matmul(st_sq[:, :], lhsT=ones_col_bf[:, :],
                         rhs=x_sq[:, kc, :], start=(kc == 0), stop=(kc == CH - 1))
    # [1, *] scalar lane: mean, var, rs = exp(-0.5*ln(var+eps)), mu*rs
    sm = stats.tile([1, 2, NT], dt.float32, tag="sm", name=f"sm_{name}")
    nc.vector.tensor_copy(out=sm[:, 0, :], in_=st_sum[:, :])
    nc.vector.tensor_copy(out=sm[:, 1, :], in_=st_sq[:, :])
    nc.vector.tensor_scalar(out=sm[:, :, :], in0=sm[:, :, :],
                            scalar1=1.0 / D, scalar2=None, op0=Alu.mult)
    var = stats.tile([1, NT], dt.float32, tag="var", name=f"var_{name}")
    nc.vector.tensor_tensor(out=var[:, :], in0=sm[:, 0, :], in1=sm[:, 0, :],
                            op=Alu.mult)
    nc.vector.tensor_tensor(out=var[:, :], in0=sm[:, 1, :], in1=var[:, :],
                            op=Alu.subtract)
    # rsmu = [rs | mu*rs] on partition 0
    rsmu = stats.tile([1, 2, NT], dt.float32, tag="rsmu", name=f"rsmu_{name}")
    nc.scalar.activation(out=var[:, :], in_=var[:, :], func=Act.Ln,
                         bias=eps_sb[:, :], scale=1.0)
    nc.scalar.activation(out=rsmu[:, 0, :], in_=var[:, :], func=Act.Exp,
                         bias=0.0, scale=-0.5)
    nc.vector.tensor_tensor(out=rsmu[:, 1, :], in0=sm[:, 0, :],
                            in1=rsmu[:, 0, :], op=Alu.mult)
    # broadcast rs and mu*rs across partitions via K=1 fp32 matmul
    bc = psum.tile([128, 2, NT], dt.float32, tag="sc", name=f"bc_{name}")
    nc.tensor.matmul(bc[:, 0, :], lhsT=ones_row_f32[:, :], rhs=rsmu[:, 0, :],
                     start=True, stop=True)
    nc.tensor.matmul(bc[:, 1, :], lhsT=ones_row_f32[:, :], rhs=rsmu[:, 1, :],
                     start=True, stop=True)
    # apply: x_out = (x*rs - mu*rs)*g + be
    t = xres.tile([128, CH, NT], dt.float32, tag="x", name=f"t_{name}")
    nc.vector.tensor_tensor(
        out=t[:, :, :], in0=x_in[:, :, :],
        in1=bc[:, 0:1, :].to_broadcast([128, CH, NT]), op=Alu.mult)
    nc.vector.tensor_tensor(
        out=t[:, :, :], in0=t[:, :, :],
        in1=bc[:, 1:2, :].to_broadcast([128, CH, NT]), op=Alu.subtract)
    x_out = xres.tile([128, CH, NT], dt.float32, tag="x", name=f"xo_{name}")
    for c in range(CH):
        nc.vector.tensor_scalar(out=x_out[:, c, :], in0=t[:, c, :],
                                scalar1=g_sb[:, c:c + 1],
                                scalar2=be_sb[:, c:c + 1],
                                op0=Alu.mult, op1=Alu.add)
    x_out_bf = xbfp.tile([128, CH, NT], dt.bfloat16, tag="xb",
                         name=f"xob_{name}")
    nc.vector.tensor_copy(out=x_out_bf[:, :, :], in_=x_out[:, :, :])
    return x_out, x_out_bf


def build_program(layers=L):
    nc = bacc.Bacc("TRN2", target_bir_lowering=False, debug=False,
                   num_devices=NC)
    # ---------------- I/O ----------------
    tok_d = nc.dram_tensor("tokens_c", [NT], dt.int32, kind="ExternalInput")
    emb_d = nc.dram_tensor("emb", [V, D], dt.float32, kind="ExternalInput")
    pe_d = nc.dram_tensor("pe_fm", [D, NT], dt.float32, kind="ExternalInput")
    wq_d = nc.dram_tensor("Wq", [layers, D, D], dt.bfloat16, kind="ExternalInput")
    wk_d = nc.dram_tensor("Wk", [layers, D, D], dt.bfloat16, kind="ExternalInput")
    wv_d = nc.dram_tensor("Wv", [layers, D, D], dt.bfloat16, kind="ExternalInput")
    wo_d = nc.dram_tensor("Wo", [layers, D, D], dt.bfloat16, kind="ExternalInput")
    w1_d = nc.dram_tensor("W1", [layers, D, F], dt.bfloat16, kind="ExternalInput")
    w2_d = nc.dram_tensor("W2", [layers, F, D], dt.bfloat16, kind="ExternalInput")
    bq_d = nc.dram_tensor("bq", [layers, D], dt.float32, kind="ExternalInput")
    bk_d = nc.dram_tensor("bk", [layers, D], dt.float32, kind="ExternalInput")
    bv_d = nc.dram_tensor("bv", [layers, D], dt.float32, kind="ExternalInput")
    bo_d = nc.dram_tensor("bo", [layers, D], dt.float32, kind="ExternalInput")
    b1_d = nc.dram_tensor("b1", [layers, F], dt.float32, kind="ExternalInput")
    b2_d = nc.dram_tensor("b2", [layers, D], dt.float32, kind="ExternalInput")
    g1_d = nc.dram_tensor("g1", [layers, D], dt.float32, kind="ExternalInput")
    be1_d = nc.dram_tensor("be1", [layers, D], dt.float32, kind="ExternalInput")
    g2_d = nc.dram_tensor("g2", [layers, D], dt.float32, kind="ExternalInput")
    be2_d = nc.dram_tensor("be2", [layers, D], dt.float32, kind="ExternalInput")
    out_d = nc.dram_tensor("out_fm", [D, NT], dt.float32, kind="ExternalOutput")

    groups = [[0, 1, 2, 3], [4, 5, 6, 7]]

    with tile.TileContext(nc) as tc, ExitStack() as ctx:
        psum = ctx.enter_context(tc.tile_pool(name="psum", bufs=2, space="PSUM"))
        consts = ctx.enter_context(tc.tile_pool(name="consts", bufs=1))
        small = ctx.enter_context(tc.tile_pool(name="small", bufs=1))
        xres = ctx.enter_context(tc.tile_pool(name="xres", bufs=3))
        xbfp = ctx.enter_context(tc.tile_pool(name="xbf", bufs=2))
        qkvp = ctx.enter_context(tc.tile_pool(name="qkv", bufs=1))
        attnp = ctx.enter_context(tc.tile_pool(name="attn", bufs=4))
        kvsp = ctx.enter_context(tc.tile_pool(name="kvs", bufs=2))
        wpool = ctx.enter_context(tc.tile_pool(name="w", bufs=3))
        hpool = ctx.enter_context(tc.tile_pool(name="h", bufs=1))
        opool = ctx.enter_context(tc.tile_pool(name="o", bufs=1))
        stats = ctx.enter_context(tc.tile_pool(name="stats", bufs=1))
        dram = ctx.enter_context(tc.tile_pool(name="dram", bufs=2, space="DRAM"))

        # ---------------- constants ----------------
        ident = consts.tile([128, 128], dt.float32, name="ident")
        from concourse.masks import make_identity
        make_identity(nc, ident[:, :])
        ones_col_bf = consts.tile([128, 1], dt.bfloat16, name="ones_col")
        nc.vector.memset(ones_col_bf[:, :], 1.0)
        ones_row_f32 = consts.tile([1, 128], dt.float32, name="ones_row")
        nc.vector.memset(ones_row_f32[:, :], 1.0)
        eps_sb = consts.tile([1, 1], dt.float32, name="eps_sb")
        nc.vector.memset(eps_sb[:, :], EPS)

        # ---------------- embedding ----------------
        tok_sb = consts.tile([128, TC], dt.int32, name="tok_sb")
        nc.sync.dma_start(out=tok_sb[:, :],
                          in_=tok_d[:].rearrange("(c p) -> p c", p=128))
        pe_sb = xres.tile([128, CH, NT], dt.float32, tag="x", name="pe_sb")
        nc.sync.dma_start(out=pe_sb[:, :, :],
                          in_=pe_d[:, :].rearrange("(c p) q -> p c q", p=128))
        x = xres.tile([128, CH, NT], dt.float32, tag="x", name="x0")
        for tc4 in range(TC):
            gath = wpool.tile([128, D], dt.float32, tag="w", name=f"gath{tc4}")
            nc.gpsimd.indirect_dma_start(
                out=gath[:, :], out_offset=None, in_=emb_d[:, :],
                in_offset=IndirectOffsetOnAxis(ap=tok_sb[:, tc4:tc4 + 1], axis=0))
            for fc in range(CH):
                tp = psum.tile([128, NT], dt.float32, tag="acc",
                               name=f"tp{tc4}_{fc}")
                nc.tensor.transpose(tp[:, 0:128],
                                    gath[:, fc * 128:(fc + 1) * 128],
                                    ident[:, :])
                nc.vector.tensor_scalar(
                    out=x[:, fc, tc4 * 128:(tc4 + 1) * 128], in0=tp[:, 0:128],
                    scalar1=float(SQRT_D), scalar2=None, op0=Alu.mult)
        nc.vector.tensor_tensor(out=x[:, :, :], in0=x[:, :, :],
                                in1=pe_sb[:, :, :], op=Alu.add)
        x_bf = xbfp.tile([128, CH, NT], dt.bfloat16, tag="xb", name="x0_bf")
        nc.vector.tensor_copy(out=x_bf[:, :, :], in_=x[:, :, :])

        # ---------------- layers ----------------
        for l in range(layers):
            wq_r = wq_d[l].rearrange("(kc p) f -> p kc f", p=128)
            wk_r = wk_d[l].rearrange("(kc p) f -> p kc f", p=128)
            wv_r = wv_d[l].rearrange("(kc p) f -> p kc f", p=128)
            wo_r = wo_d[l].rearrange("(kc p) f -> p kc f", p=128)
            w1_r = w1_d[l].rearrange("(kc p) f -> p kc f", p=128)
            w2_r = w2_d[l].rearrange("(kc p) f -> p kc f", p=128)

            bq_sb = small.tile([128, CH], dt.float32, tag="bq", name=f"bq{l}")
            nc.sync.dma_start(out=bq_sb[:, :],
                              in_=bq_d[l].rearrange("(c p) -> p c", p=128))
            bk_sb = small.tile([128, CH], dt.float32, tag="bk", name=f"bk{l}")
            nc.sync.dma_start(out=bk_sb[:, :],
                              in_=bk_d[l].rearrange("(c p) -> p c", p=128))
            bo_sb = small.tile([128, CH], dt.float32, tag="bo", name=f"bo{l}")
            nc.sync.dma_start(out=bo_sb[:, :],
                              in_=bo_d[l].rearrange("(c p) -> p c", p=128))
            b2_sb = small.tile([128, CH], dt.float32, tag="b2", name=f"b2{l}")
            nc.sync.dma_start(out=b2_sb[:, :],
                              in_=b2_d[l].rearrange("(c p) -> p c", p=128))
            b1_sb = small.tile([128, FCH], dt.float32, tag="b1", name=f"b1{l}")
            nc.sync.dma_start(out=b1_sb[:, :],
                              in_=b1_d[l].rearrange("(c p) -> p c", p=128))
            g1_sb = small.tile([128, CH], dt.float32, tag="g1", name=f"g1{l}")
            nc.sync.dma_start(out=g1_sb[:, :],
                              in_=g1_d[l].rearrange("(c p) -> p c", p=128))
            be1_sb = small.tile([128, CH], dt.float32, tag="be1", name=f"be1{l}")
            nc.sync.dma_start(out=be1_sb[:, :],
                              in_=be1_d[l].rearrange("(c p) -> p c", p=128))
            g2_sb = small.tile([128, CH], dt.float32, tag="g2", name=f"g2{l}")
            nc.sync.dma_start(out=g2_sb[:, :],
                              in_=g2_d[l].rearrange("(c p) -> p c", p=128))
            be2_sb = small.tile([128, CH], dt.float32, tag="be2", name=f"be2{l}")
            nc.sync.dma_start(out=be2_sb[:, :],
                              in_=be2_d[l].rearrange("(c p) -> p c", p=128))
            bv_l = bv_d[l]
            bv_bc = small.tile([128, D], dt.float32, tag="bv", name=f"bv{l}")
            nc.gpsimd.dma_start(
                out=bv_bc[:, :],
                in_=bass.AP(tensor=bv_l.tensor, offset=bv_l.offset,
                            ap=[[0, 128]] + list(bv_l.ap)))

            # ---- Q/K projections (feature-major outputs) ----
            # Q is stored [64 head-feats, H, NT] so each head starts at
            # base partition 0 (matmul requires lhsT/rhs base match).
            q_bf = qkvp.tile([64, H, NT], dt.bfloat16, tag="q", name=f"q{l}")
            k_bf = qkvp.tile([128, CH, NT], dt.bfloat16, tag="k", name=f"k{l}")
            for m in range(CH):
                wm = wpool.tile([128, CH, 128], dt.bfloat16, tag="w",
                                name=f"wq{l}_{m}")
                nc.sync.dma_start(out=wm[:, :, :],
                                  in_=wq_r[:, :, m * 128:(m + 1) * 128])
                ps = psum.tile([128, NT], dt.float32, tag="acc",
                               name=f"psq{l}_{m}")
                for kc in range(CH):
                    nc.tensor.matmul(ps[:, :], lhsT=wm[:, kc, :],
                                     rhs=x_bf[:, kc, :],
                                     start=(kc == 0), stop=(kc == CH - 1))
                nc.vector.tensor_scalar(out=q_bf[:, 2 * m, :], in0=ps[0:64, :],
                                        scalar1=bq_sb[0:64, m:m + 1],
                                        scalar2=None, op0=Alu.add)
                nc.vector.tensor_scalar(out=q_bf[:, 2 * m + 1, :],
                                        in0=ps[64:128, :],
                                        scalar1=bq_sb[64:128, m:m + 1],
                                        scalar2=None, op0=Alu.add)
            for m in range(CH):
                wm = wpool.tile([128, CH, 128], dt.bfloat16, tag="w",
                                name=f"wk{l}_{m}")
                nc.sync.dma_start(out=wm[:, :, :],
                                  in_=wk_r[:, :, m * 128:(m + 1) * 128])
                ps = psum.tile([128, NT], dt.float32, tag="acc",
                               name=f"psk{l}_{m}")
                for kc in range(CH):
                    nc.tensor.matmul(ps[:, :], lhsT=wm[:, kc, :],
                                     rhs=x_bf[:, kc, :],
                                     start=(kc == 0), stop=(kc == CH - 1))
                nc.vector.tensor_scalar(out=k_bf[:, m, :], in0=ps[:, :],
                                        scalar1=bk_sb[:, m:m + 1],
                                        scalar2=None, op0=Alu.add)

            # ---- V projection (token-major output) ----
            v_bf = qkvp.tile([128, TC, D], dt.bfloat16, tag="v", name=f"v{l}")
            for n in range(2):
                wvn = wpool.tile([128, CH, 512], dt.bfloat16, tag="w",
                                 name=f"wv{l}_{n}")
                nc.sync.dma_start(out=wvn[:, :, :],
                                  in_=wv_r[:, :, n * 512:(n + 1) * 512])
                for tc4 in range(TC):
                    ps = psum.tile([128, 512], dt.float32, tag="acc",
                                   name=f"psv{l}_{n}_{tc4}")
                    for kc in range(CH):
                        nc.tensor.matmul(
                            ps[:, :],
                            lhsT=x_bf[:, kc, tc4 * 128:(tc4 + 1) * 128],
                            rhs=wvn[:, kc, :],
                            start=(kc == 0), stop=(kc == CH - 1))
                    nc.vector.tensor_tensor(
                        out=v_bf[:, tc4, n * 512:(n + 1) * 512], in0=ps[:, :],
                        in1=bv_bc[:, n * 512:(n + 1) * 512], op=Alu.add)

            # ---- K/V AllGather across the 4 cores of this batch ----
            kv_own = dram.tile([2 * KV_HALF], dt.bfloat16, tag="kvo",
                               name=f"kvo{l}")
            nc.sync.dma_start(
                out=kv_own[0:KV_HALF].rearrange("(c p q) -> p c q", p=128, q=NT),
                in_=k_bf[:, :, :])
            nc.sync.dma_start(
                out=kv_own[KV_HALF:].rearrange("(t p f) -> p t f", p=128, f=D),
                in_=v_bf[:, :, :])
            kv_g = dram.tile([4, 2 * KV_HALF], dt.bfloat16, tag="kvg",
                             name=f"kvg{l}")
            nc.gpsimd.collective_compute(
                "AllGather", Alu.bypass, replica_groups=groups,
                ins=[kv_own[:].opt()], outs=[kv_g[:, :].opt()])

            kga = kv_g[:, 0:KV_HALF].rearrange("g (c p q) -> c p g q",
                                               p=128, q=NT)
            vga = kv_g[:, KV_HALF:].rearrange("g (t p f) -> p g t f",
                                              p=128, f=D)

            # ---- attention ----
            o_nbf = opool.tile([128, CH, NT], dt.bfloat16, tag="onb",
                               name=f"onb{l}")
            for h in range(H):
                pq = (h % 2) * 64
                k_sb = kvsp.tile([64, 4, NT], dt.bfloat16, tag="k",
                                 name=f"ksb{l}_{h}")
                nc.sync.dma_start(out=k_sb[:, :, :],
                                  in_=kga[h // 2, pq:pq + 64, :, :])
                v_sb = kvsp.tile([128, KCH, 65], dt.bfloat16, tag="v",
                                 name=f"vsb{l}_{h}")
                nc.vector.memset(v_sb[:, :, 64:65], 1.0)
                for g in range(4):
                    nc.sync.dma_start(
                        out=v_sb[:, 4 * g:4 * (g + 1), 0:64],
                        in_=vga[:, g, :, h * 64:(h + 1) * 64])
                av = psum.tile([128, NT], dt.float32, tag="av", name=f"av{l}_{h}")
                for w in range(KCH // 2):
                    sc = psum.tile([128, 2, NT], dt.float32, tag="sc",
                                   name=f"sc{l}_{h}_{w}")
                    at = attnp.tile([128, 2, NT], dt.bfloat16, tag="at",
                                    name=f"at{l}_{h}_{w}")
                    for i in range(2):
                        kc = 2 * w + i
                        nc.tensor.matmul(
                            sc[:, i, :],
                            lhsT=k_sb[:, kc // 4, (kc % 4) * 128:(kc % 4 + 1) * 128],
                            rhs=q_bf[:, h, :],
                            start=True, stop=True)
                    nc.scalar.activation(out=at[:, :, :], in_=sc[:, :, :],
                                         func=Act.Exp, bias=0.0, scale=0.125)
                    for i in range(2):
                        kc = 2 * w + i
                        nc.tensor.matmul(av[0:65, :], lhsT=v_sb[:, kc, :],
                                         rhs=at[:, i, :], start=(kc == 0),
                                         stop=(kc == KCH - 1),
                                         skip_group_check=True)
                # normalize by the softmax denominator (row 64 of av)
                o_h = attnp.tile([64, NT], dt.bfloat16, tag="oh",
                                 name=f"oh{l}_{h}")
                nc.vector.tensor_copy(out=o_h[:, :], in_=av[0:64, :])
                zi_h = stats.tile([1, NT], dt.float32, tag="zi",
                                  name=f"zi{l}_{h}")
                nc.vector.reciprocal(out=zi_h[:, :], in_=av[64:65, :])
                bz = psum.tile([64, NT], dt.float32, tag="acc",
                               name=f"bz{l}_{h}")
                nc.tensor.matmul(bz[:, :], lhsT=ones_row_f32[0:1, 0:64],
                                 rhs=zi_h[:, :], start=True, stop=True)
                nc.vector.tensor_tensor(out=o_nbf[pq:pq + 64, h // 2, :],
                                        in0=o_h[:, :], in1=bz[:, :],
                                        op=Alu.mult)

            # ---- Wo projection + residual ----
            x1 = xres.tile([128, CH, NT], dt.float32, tag="x", name=f"x1_{l}")
            for m in range(CH):
                wm = wpool.tile([128, CH, 128], dt.bfloat16, tag="w",
                                name=f"wo{l}_{m}")
                nc.sync.dma_start(out=wm[:, :, :],
                                  in_=wo_r[:, :, m * 128:(m + 1) * 128])
                ps = psum.tile([128, NT], dt.float32, tag="acc",
                               name=f"pso{l}_{m}")
                for kc in range(CH):
                    nc.tensor.matmul(ps[:, :], lhsT=wm[:, kc, :],
                                     rhs=o_nbf[:, kc, :],
                                     start=(kc == 0), stop=(kc == CH - 1))
                nc.vector.tensor_scalar(out=x1[:, m, :], in0=ps[:, :],
                                        scalar1=bo_sb[:, m:m + 1],
                                        scalar2=None, op0=Alu.add)
            nc.vector.tensor_tensor(out=x1[:, :, :], in0=x1[:, :, :],
                                    in1=x[:, :, :], op=Alu.add)

            x1n, x1n_bf = _ln(ctx, nc, psum, xres, xbfp, stats, x1, g1_sb,
                              be1_sb, ones_col_bf, ones_row_f32, eps_sb,
                              f"l{l}a")

            # ---- FFN ----
            x2 = xres.tile([128, CH, NT], dt.float32, tag="x", name=f"x2_{l}")
            for half in range(2):
                h_bf = hpool.tile([128, FCH // 2, NT], dt.bfloat16, tag="h",
                                  name=f"h{l}_{half}")
                for mi in range(FCH // 2):
                    m = half * (FCH // 2) + mi
                    w1m = wpool.tile([128, CH, 128], dt.bfloat16, tag="w",
                                     name=f"w1_{l}_{m}")
                    nc.sync.dma_start(out=w1m[:, :, :],
                                      in_=w1_r[:, :, m * 128:(m + 1) * 128])
                    ps = psum.tile([128, NT], dt.float32, tag="acc",
                                   name=f"ps1{l}_{m}")
                    for kc in range(CH):
                        nc.tensor.matmul(ps[:, :], lhsT=w1m[:, kc, :],
                                         rhs=x1n_bf[:, kc, :],
                                         start=(kc == 0), stop=(kc == CH - 1))
                    nc.vector.tensor_scalar(out=h_bf[:, mi, :], in0=ps[:, :],
                                            scalar1=b1_sb[:, m:m + 1],
                                            scalar2=0.0, op0=Alu.add,
                                            op1=Alu.max)
                for m in range(CH):
                    w2m = wpool.tile([128, FCH // 2, 128], dt.bfloat16, tag="w",
                                     name=f"w2_{l}_{half}_{m}")
                    nc.sync.dma_start(
                        out=w2m[:, :, :],
                        in_=w2_r[:, half * (FCH // 2):(half + 1) * (FCH // 2),
                                 m * 128:(m + 1) * 128])
                    ps = psum.tile([128, NT], dt.float32, tag="acc",
                                   name=f"ps2{l}_{half}_{m}")
                    for kc in range(FCH // 2):
                        nc.tensor.matmul(ps[:, :], lhsT=w2m[:, kc, :],
                                         rhs=h_bf[:, kc, :],
                                         start=(kc == 0),
                                         stop=(kc == FCH // 2 - 1))
                    if half == 0:
                        nc.vector.tensor_scalar(out=x2[:, m, :], in0=ps[:, :],
                                                scalar1=b2_sb[:, m:m + 1],
                                                scalar2=None, op0=Alu.add)
                    else:
                        nc.vector.tensor_tensor(out=x2[:, m, :],
                                                in0=x2[:, m, :], in1=ps[:, :],
                                                op=Alu.add)
            nc.vector.tensor_tensor(out=x2[:, :, :], in0=x2[:, :, :],
                                    in1=x1n[:, :, :], op=Alu.add)

            x, x_bf = _ln(ctx, nc, psum, xres, xbfp, stats, x2, g2_sb, be2_sb,
                          ones_col_bf, ones_row_f32, eps_sb, f"l{l}b")

        # ---------------- output ----------------
        nc.sync.dma_start(
            out=out_d[:, :].rearrange("(c p) q -> p c q", p=128),
            in_=x[:, :, :])

    nc.compile()
    return nc


_PROG = {}


def _get_prog(layers=L):
    if layers not in _PROG:
        _PROG[layers] = build_program(layers)
    return _PROG[layers]


def _host_inputs(inputs, layers=L):
    """Build the 8 per-core input maps from the full-model inputs."""
    bf16 = ml_dtypes.bfloat16
    f32 = np.float32
    tokens = np.asarray(inputs["tokens"])
    # positional encoding (constant)
    pos = np.arange(S)[:, None].astype(f32)
    freq = np.exp(np.arange(0, D, 2).astype(f32) * -(math.log(10000.0) / D))
    pe = np.zeros((S, D), dtype=f32)
    pe[:, 0::2] = np.sin(pos * freq)
    pe[:, 1::2] = np.cos(pos * freq)
    shared = {
        "emb": np.ascontiguousarray(np.asarray(inputs["emb"], dtype=f32)),
    }
    for nm in ("Wq", "Wk", "Wv", "Wo", "W1", "W2"):
        shared[nm] = np.ascontiguousarray(
            np.asarray(inputs[nm])[:layers].astype(bf16))
    for nm in ("bq", "bk", "bv", "bo", "b1", "b2", "g1", "be1", "g2", "be2"):
        shared[nm] = np.ascontiguousarray(
            np.asarray(inputs[nm])[:layers].astype(f32))

    in_maps = []
    for core in range(NC):
        b, lc = core // 4, core % 4
        m = dict(shared)
        m["tokens_c"] = np.ascontiguousarray(tokens[b, lc * NT:(lc + 1) * NT])
        m["pe_fm"] = np.ascontiguousarray(pe[lc * NT:(lc + 1) * NT, :].T)
        in_maps.append(m)
    return in_maps


def run(inputs, layers=L, trace=False):
    nc = _get_prog(layers)
    in_maps = _host_inputs(inputs, layers)
    res = run_bass_kernel_spmd(nc, in_maps, list(range(NC)), trace=trace)
    out = np.zeros((B, S, D), dtype=np.float32)
    for core in range(NC):
        b, lc = core // 4, core % 4
        out[b, lc * NT:(lc + 1) * NT, :] = res.results[core]["out_fm"].T
    return out, res


def kernel(**inputs):
    out, _ = run(inputs)
    return out
